# revision 1
# baseline (speedup 1.0000x reference)
"""Trainium2 Bass kernel for a 12-head attention module (B=4, S=1024, E=256, H=12,
per-head dim = E — the module quirk that makes per-head weight fusion possible).

Sharding: 8 cores = 4 batches x 2 head-groups (6 heads each).  Each core computes
its partial fc projection; the host sums the two partials per batch element
(the "all-reduce after fc" from the sharding hint, done host-side since the
partial-sum add is tiny compared to the attention compute).

Algebraic fusion (host precomputes, in float64):
  A_h^T = scale * Wk_h @ Wq_h^T   (E x E)  ->  scoresT_h = (A_h @ x^T)^T ... i.e.
      uT_h    = A_h @ x^T            [E, S]    (lhsT = A_h^T, rhs = xT)
      scoresT = uT_h^T-contracted    [s_k, s_q] (lhsT = uT_h ki-block, rhs = xT)
  so q/k projections collapse into ONE projection and kT/qT never exist.
  Nonzero bq/bk reduce to a per-key bias on the exp (q-side terms are constant
  along the softmax axis and cancel); see s2 below.

  C_h = Wv_h @ Wfc_h   (E x E)  ->  the fc layer disappears:
      w_h  = x @ C_h                 [S, E]    (lhsT = xT s-block, rhs = C_h)
      out  = sum_h softmax(scores_h) @ w_h
  bv and bfc become an exact host-side constant row:  out += bv @ Wfc + bfc.

  A ones column appended to w_h (N=258, padded even for fp32r) makes the ctx
  matmul emit softmax denominators in PSUM column 256 for free — no separate
  rowsum matmuls.

Softmax skips the max-subtraction: scores are O(0.5) so exp is safe, and masked
entries underflow to exactly 0.0 in fp32, identical to the reference's
exp(masked - rowmax).  The additive mask is applied as a multiplicative
exp(mask) factor; the host classifies 128x128 blocks of exp(mask^T) into
all-zero (skipped entirely), all-one (no-op), and mixed (multiplied on-device),
which discovers the causal structure automatically.

All matmul inputs are float32r (TF32-class PE mode: full speed at free-dim
>= 256, ~4x faster than fp32).
"""

import numpy as np

import concourse.mybir as mybir
import concourse.tile as tile
from concourse import bacc
from concourse.bass_utils import run_bass_kernel_spmd

# Problem constants
B, S, E, H = 4, 1024, 256, 12
P = 128
NCORES = 8
HPC = H // 2            # heads per core
EH = E * HPC            # 1536 = per-core head width
KS_E = E // P           # 2 contraction subtiles over E
ST = S // P             # 8 row-blocks of S
EW = E + 2              # w width incl. ones column (+pad: fp32r needs even free dims)

MM_DT = mybir.dt.float32r

LAST_RESULTS = None     # BassKernelResults of the most recent run (for harness)


def _chunks(w):
    """Split width w (multiple of 128) into matmul free-dim chunks <=512,
    preferring >=256 (float32r runs 4x slower below 256)."""
    out = []
    while w > 0:
        if w >= 768:
            c = 512
        elif w == 640:
            c = 384
        else:
            c = min(w, 512)
        out.append(c)
        w -= c
    return out


def _mask_structure(attention_mask):
    """Classify 128x128 blocks of exp(mask^T) -> (structure, unique_blocks)."""
    m = np.asarray(attention_mask, dtype=np.float64).reshape(S, S)   # [q, k]
    em = np.exp(m).astype(np.float32)
    emT = np.ascontiguousarray(em.T)                                 # [k, q]

    uniq: dict[bytes, int] = {}
    blocks = {}
    for ki in range(ST):
        for qj in range(ST):
            blk = np.ascontiguousarray(emT[ki * P:(ki + 1) * P, qj * P:(qj + 1) * P])
            if not blk.any():
                blocks[(ki, qj)] = "skip"
            elif (blk == 1.0).all():
                blocks[(ki, qj)] = "one"
            else:
                blocks[(ki, qj)] = uniq.setdefault(blk.tobytes(), len(uniq))

    zkey = np.zeros((P, P), np.float32).tobytes()
    spans = []
    for ki in range(ST):
        non = [qj for qj in range(ST) if blocks[(ki, qj)] != "skip"]
        if not non:
            spans.append(None)
            continue
        qa, qb = non[0] * P, (non[-1] + 1) * P
        if qb - qa == P:
            # widen 128-wide spans to 256: fp32r runs 4x slower below N=256,
            # so one extra (masked-to-zero) block is cheaper than a slow chunk
            if qa >= P:
                qa -= P
            elif qb + P <= S:
                qb += P
        spans.append((qa, qb))

    # every non-"one" block inside a span needs a multiply (interior skips too)
    mixed = []
    for ki in range(ST):
        if spans[ki] is None:
            continue
        qa, qb = spans[ki]
        for qj in range(qa // P, qb // P):
            bl = blocks[(ki, qj)]
            if bl == "one":
                continue
            if bl == "skip":
                bl = uniq.setdefault(zkey, len(uniq))
            mixed.append((ki, qj, bl))

    # per q-block m: key row-blocks ki whose span covers block m
    covers = []
    for m_ in range(ST):
        ks = tuple(ki for ki in range(ST)
                   if spans[ki] is not None
                   and spans[ki][0] <= m_ * P and spans[ki][1] >= (m_ + 1) * P)
        assert ks, (
            "attention row-block with no unmasked keys is not supported "
            "(reference softmax of an all-masked row is uniform)")
        covers.append(ks)

    nuniq = max(len(uniq), 1)
    ublocks = np.zeros((nuniq, P, P), np.float32)
    for key, uid in uniq.items():
        ublocks[uid] = np.frombuffer(key, np.float32).reshape(P, P)

    struct = (tuple(spans), tuple(mixed), tuple(covers), nuniq)
    return struct, ublocks


def _build(struct, mm_dt, reps=1):
    spans, mixed, covers, nuniq, has_qk_bias = struct
    f32 = mybir.dt.float32
    Exp = mybir.ActivationFunctionType.Exp

    # packed probsT column offsets per ki
    probs_off = []
    tot = 0
    for ki in range(ST):
        probs_off.append(tot)
        if spans[ki] is not None:
            tot += spans[ki][1] - spans[ki][0]
    mixed_by_ki = {}
    for ki, qj, uid in mixed:
        mixed_by_ki.setdefault(ki, []).append((qj, uid))

    nc = bacc.Bacc("TRN2")
    xT_d = nc.dram_tensor("xT", (E, S), mm_dt, kind="ExternalInput")
    wa_d = nc.dram_tensor("wa", (E, EH), mm_dt, kind="ExternalInput")
    wc_d = nc.dram_tensor("wc", (E, EH), mm_dt, kind="ExternalInput")
    wm2_d = nc.dram_tensor("wm2", (E, 2 * HPC), mm_dt, kind="ExternalInput")
    em_d = nc.dram_tensor("emask", (nuniq, P, P), mm_dt, kind="ExternalInput")
    ones_d = nc.dram_tensor("ones", (P, P), mm_dt, kind="ExternalInput")
    y_d = nc.dram_tensor("y", (S, E), f32, kind="ExternalOutput")

    with tile.TileContext(nc) as tc, \
            tc.tile_pool(name="singles", bufs=1) as singles, \
            tc.tile_pool(name="heads", bufs=2) as heads, \
            tc.tile_pool(name="small", bufs=4) as small, \
            tc.tile_pool(name="psA", bufs=6, space="PSUM") as psA, \
            tc.tile_pool(name="psC", bufs=2, space="PSUM") as psC:

        # ---- resident tensors, DMA'd in first-use order ----
        xT_sb = singles.tile([P, KS_E, S], mm_dt)
        xT_r = xT_d[:, :].rearrange("(ko p) n -> p ko n", p=P)
        wa_sb = singles.tile([P, KS_E, EH], mm_dt)
        wc_sb = singles.tile([P, KS_E, EH], mm_dt)
        h0 = slice(0, E)
        wa_r = wa_d[:, h0].rearrange("(ko p) n -> p ko n", p=P)
        wc_r = wc_d[:, h0].rearrange("(ko p) n -> p ko n", p=P)
        for ks in range(KS_E):
            nc.sync.dma_start(wa_sb[:, ks, h0], wa_r[:, ks, :])
            nc.gpsimd.dma_start(xT_sb[:, ks, 0:512], xT_r[:, ks, 0:512])
        nc.sync.dma_start(wc_sb[:, :, h0], wc_r)
        for ks in range(KS_E):
            nc.gpsimd.dma_start(xT_sb[:, ks, 512:1024], xT_r[:, ks, 512:1024])
        em_sb = singles.tile([P, nuniq, P], mm_dt)
        nc.gpsimd.dma_start(em_sb, em_d[:, :, :].rearrange("u p q -> p u q"))
        wm2_sb = None
        if has_qk_bias:
            wm2_sb = singles.tile([P, KS_E, 2 * HPC], mm_dt)
            nc.sync.dma_start(
                wm2_sb, wm2_d[:, :].rearrange("(ko p) n -> p ko n", p=P))
        for h in range(1, HPC):
            hs = slice(h * E, (h + 1) * E)
            for sb, d in ((wa_sb, wa_d), (wc_sb, wc_d)):
                nc.sync.dma_start(
                    sb[:, :, hs], d[:, hs].rearrange("(ko p) n -> p ko n", p=P))
        acc_sb = singles.tile([P, ST, E], f32)

        for _rep in range(reps):
            for h in range(HPC):
                # ---- uT_h = A_h @ x^T  [E(2 tiles), S] ----
                uT = heads.tile([P, KS_E, S], mm_dt, tag="uT")
                for jn in range(S // 512):
                    for t in range(KS_E):
                        ps = psA.tile([P, 512], f32, tag="mm512", name="ps_u")
                        for ks in range(KS_E):
                            nc.tensor.matmul(
                                ps,
                                wa_sb[:, ks, h * E + t * P: h * E + (t + 1) * P],
                                xT_sb[:, ks, jn * 512:(jn + 1) * 512],
                                start=(ks == 0), stop=(ks == KS_E - 1),
                            )
                        nc.scalar.copy(uT[:, t, jn * 512:(jn + 1) * 512], ps)

                # ---- w_h = x @ C_h  [S(8 blocks), E] + ones column ----
                ww = heads.tile([P, ST, EW], mm_dt, tag="w")
                nc.sync.dma_start(
                    ww[:, :, E:EW],
                    ones_d[:, 0:ST * 2].rearrange("p (a b) -> p a b", b=2))
                for st in range(ST):
                    ps = psA.tile([P, 512], f32, tag="mm512", name="ps_w")[:, :E]
                    for ks in range(KS_E):
                        nc.tensor.matmul(
                            ps,
                            xT_sb[:, ks, st * P:(st + 1) * P],
                            wc_sb[:, ks, h * E:(h + 1) * E],
                            start=(ks == 0), stop=(ks == KS_E - 1),
                        )
                    nc.vector.tensor_copy(ww[:, st, :E], ps)

                # ---- s2_h = x @ (scale * Wk_h @ bq_h): per-key exp bias ----
                s2 = None
                if has_qk_bias:
                    s2 = heads.tile([P, ST, 2], f32, tag="s2")
                    for st in range(ST):
                        ps = psA.tile([P, 512], f32, tag="mm512", name="ps_s2")[:, :2]
                        for ks in range(KS_E):
                            nc.tensor.matmul(
                                ps,
                                xT_sb[:, ks, st * P:(st + 1) * P],
                                wm2_sb[:, ks, 2 * h:2 * h + 2],
                                start=(ks == 0), stop=(ks == KS_E - 1),
                            )
                        nc.vector.tensor_copy(s2[:, st, :], ps)

                # ---- scoresT -> exp -> (mask multiply) => probsT (packed) ----
                probs = heads.tile([P, tot], mm_dt, tag="probs", bufs=2)
                for ki in range(ST):
                    if spans[ki] is None:
                        continue
                    qa, qb = spans[ki]
                    off = probs_off[ki]
                    pos = qa
                    for w in _chunks(qb - qa):
                        ps = psA.tile([P, 512], f32, tag="mm512", name="ps_s")[:, :w]
                        for ks in range(KS_E):
                            nc.tensor.matmul(
                                ps,
                                uT[:, ks, ki * P:(ki + 1) * P],
                                xT_sb[:, ks, pos:pos + w],
                                start=(ks == 0), stop=(ks == KS_E - 1),
                            )
                        if has_qk_bias:
                            nc.scalar.activation(
                                probs[:, off + pos - qa: off + pos - qa + w], ps,
                                Exp, bias=s2[:, ki, 0:1])
                        else:
                            nc.scalar.activation(
                                probs[:, off + pos - qa: off + pos - qa + w], ps, Exp)
                        pos += w
                    for qj, uid in mixed_by_ki.get(ki, ()):
                        sl = slice(off + qj * P - qa, off + (qj + 1) * P - qa)
                        nc.gpsimd.tensor_mul(probs[:, sl], probs[:, sl],
                                             em_sb[:, uid, :])

                # ---- out block m: sum_ki probsT(ki,m)^T @ [w_h | 1] -> [128, 257]
                #      col 256 = softmax denominator; normalize + accumulate ----
                for m_ in range(ST):
                    ks_list = covers[m_]
                    ps = psC.tile([P, EW], f32, tag="ctx", name="ps_c")
                    last = len(ks_list) - 1
                    for idx, ki in enumerate(ks_list):
                        qa, _ = spans[ki]
                        off = probs_off[ki]
                        nc.tensor.matmul(
                            ps,
                            probs[:, off + m_ * P - qa: off + (m_ + 1) * P - qa],
                            ww[:, ki, :],
                            start=(idx == 0), stop=(idx == last),
                        )
                    rec = small.tile([P, 1], f32, tag="rec")
                    nc.vector.reciprocal(rec, ps[:, E:E + 1])
                    if h == 0:
                        nc.vector.tensor_scalar_mul(acc_sb[:, m_, :], ps[:, :E], rec)
                    else:
                        tmp = small.tile([P, E], f32, tag="tmp")
                        nc.vector.tensor_scalar_mul(tmp, ps[:, :E], rec)
                        nc.gpsimd.tensor_add(acc_sb[:, m_, :], acc_sb[:, m_, :], tmp)
                    if h == HPC - 1:
                        nc.sync.dma_start(y_d[m_ * P:(m_ + 1) * P, :], acc_sb[:, m_, :])

    nc.compile()   # bacc passes: split sync waits, move matmul waits to ldweights
    return nc


_nc_cache = {}


def kernel(x, attention_mask, Wq, bq, Wk, bk, Wv, bv, Wfc, bfc, _trace=False):
    global LAST_RESULTS
    x = np.asarray(x, np.float32)
    Wq64 = np.asarray(Wq, np.float64)
    Wk64 = np.asarray(Wk, np.float64)
    Wv64 = np.asarray(Wv, np.float64)
    Wfc64 = np.asarray(Wfc, np.float64)
    bq64 = np.asarray(bq, np.float64)
    bk64 = np.asarray(bk, np.float64)
    bv64 = np.asarray(bv, np.float64)
    bfc = np.asarray(bfc, np.float32)

    has_qk_bias = bool(bq64.any())
    struct, ublocks = _mask_structure(attention_mask)
    struct = struct + (has_qk_bias,)
    key = (struct, str(MM_DT))
    if key not in _nc_cache:
        _nc_cache[key] = _build(struct, MM_DT)
    nc = _nc_cache[key]

    scale = 1.0 / np.sqrt(np.float64(E))
    # per-head fused weights (float64 on host, cast to fp32)
    # A_h^T = scale * Wk_h @ Wq_h^T ; C_h = Wv_h @ Wfc_h ;
    # m2_h = scale * Wk_h @ bq_h  (per-key score bias; q-side terms cancel)
    wa = np.empty((E, E * H), np.float32)
    wc = np.empty((E, E * H), np.float32)
    wm2 = np.zeros((E, 2 * H), np.float32)
    for g in range(H):
        gs = slice(g * E, (g + 1) * E)
        wa[:, gs] = (scale * (Wk64[:, gs] @ Wq64[:, gs].T)).astype(np.float32)
        wc[:, gs] = (Wv64[:, gs] @ Wfc64[gs, :]).astype(np.float32)
        wm2[:, 2 * g] = (scale * (Wk64[:, gs] @ bq64[gs])).astype(np.float32)
    # exact host-side output constant: bk only shifts scores along the softmax
    # axis's constant direction...: k-side bias enters via m2; v/fc biases:
    ybias = (bv64 @ Wfc64 + np.asarray(bfc, np.float64)).astype(np.float32)

    in_maps = []
    for c in range(NCORES):
        b, hg = divmod(c, 2)
        cs = slice(hg * EH, (hg + 1) * EH)
        in_maps.append({
            "xT": np.ascontiguousarray(x[b].T),
            "wa": np.ascontiguousarray(wa[:, cs]),
            "wc": np.ascontiguousarray(wc[:, cs]),
            "wm2": np.ascontiguousarray(wm2[:, hg * 2 * HPC:(hg + 1) * 2 * HPC]),
            "emask": ublocks,
            "ones": np.ones((P, P), np.float32),
        })

    from concourse._compat import axon_active
    if axon_active() and not _trace:
        results = _run_pjrt_cached(key, nc, in_maps)
        LAST_RESULTS = None
    else:
        try:
            res = run_bass_kernel_spmd(nc, in_maps, core_ids=list(range(NCORES)),
                                       trace=_trace)
        except ModuleNotFoundError:
            # axon client without NTFF-profiling support: tracing disabled
            import os
            os.environ["BASS_NEVER_TRACE"] = "1"
            res = run_bass_kernel_spmd(nc, in_maps, core_ids=list(range(NCORES)),
                                       trace=False)
        LAST_RESULTS = res
        results = res.results
    out = np.empty((B, S, E), np.float32)
    for b in range(B):
        out[b] = results[2 * b]["y"] + results[2 * b + 1]["y"] + ybias
    return out


_jit_cache = {}


def _run_pjrt_cached(key, nc, in_maps):
    """bass2jax.run_bass_via_pjrt with the sharded jit cached per kernel
    structure, so repeated kernel() calls skip re-tracing (and with it the
    expensive NEFF recompile inside the neuronx_cc hook)."""
    import jax
    from jax.sharding import Mesh, PartitionSpec
    from jax.experimental.shard_map import shard_map
    from concourse import bass2jax
    import concourse.mybir as _mybir

    if key not in _jit_cache:
        bass2jax.install_neuronx_cc_hook()
        in_names, out_names, out_avals, zero_shapes = [], [], [], []
        for alloc in nc.m.functions[0].allocations:
            if not isinstance(alloc, _mybir.MemoryLocationSet):
                continue
            name = alloc.memorylocations[0].name
            if alloc.kind == "ExternalInput":
                if name != "partition_id":
                    in_names.append(name)
            elif alloc.kind == "ExternalOutput":
                shape = tuple(alloc.tensor_shape)
                dtype = _mybir.dt.np(alloc.dtype)
                out_names.append(name)
                out_avals.append(jax.core.ShapedArray(shape, dtype))
                zero_shapes.append((shape, dtype))
        n_params = len(in_names)
        n_outs = len(out_names)
        all_names = in_names + out_names + ["partition_id"]

        def _body(*args):
            operands = list(args)
            operands.append(bass2jax.partition_id_tensor())
            return tuple(bass2jax._bass_exec_p.bind(
                *operands,
                out_avals=tuple(out_avals),
                in_names=tuple(all_names),
                out_names=tuple(out_names),
                lowering_input_output_aliases=(),
                sim_require_finite=True,
                sim_require_nnan=True,
                nc=nc,
            ))

        devices = jax.devices()[:NCORES]
        mesh = Mesh(np.asarray(devices), ("core",))
        sharded = jax.jit(
            shard_map(_body, mesh=mesh,
                      in_specs=(PartitionSpec("core"),) * (n_params + n_outs),
                      out_specs=(PartitionSpec("core"),) * n_outs,
                      check_rep=False),
            donate_argnums=tuple(range(n_params, n_params + n_outs)),
            keep_unused=True,
        )
        _jit_cache[key] = (sharded, in_names, out_names, out_avals, zero_shapes)

    sharded, in_names, out_names, out_avals, zero_shapes = _jit_cache[key]
    concat_in = [
        np.concatenate([np.asarray(m[name]) for m in in_maps], axis=0)
        for name in in_names
    ]

    def _exec():
        concat_zeros = [np.zeros((NCORES * s[0], *s[1:]), d)
                        for s, d in zero_shapes]
        out_arrs = sharded(*concat_in, *concat_zeros)
        return [np.asarray(a) for a in out_arrs]

    try:
        out_arrs = _exec()
    except Exception:
        # transient device/transport flake: drop the failed call's effect
        # tokens (else jax's atexit block_until_ready re-raises even after a
        # successful retry) and retry once with fresh buffers
        try:
            from jax._src import dispatch as _jd
            _jd.runtime_tokens.clear()
        except Exception:
            pass
        out_arrs = _exec()
    return [
        {name: out_arrs[i].reshape(NCORES, *out_avals[i].shape)[c]
         for i, name in enumerate(out_names)}
        for c in range(NCORES)
    ]



# revision 6
# speedup vs baseline: 1.2974x; 1.2974x over previous
"""Trainium2 Bass kernel for a 12-head attention module (B=4, S=1024, E=256, H=12,
per-head dim = E — the module quirk that makes per-head weight fusion possible).

Sharding: 8 cores = 4 batches x 2 head-groups (6 heads each).  Each core computes
its partial fc projection; the host sums the two partials per batch element.

Algebraic fusion (host precomputes, in float64):
  wa_h = scale * Wk_h @ Wq_h^T  (so uT = wa^T @ xT and scoresT = uT^T-contracted xT:
  the q/k projections collapse into one matmul chain and qT/kT never exist).
  wc_h = Wv_h @ Wfc_h  (the fc layer disappears: w_h = x @ wc_h,
  out = sum_h softmax(scores_h) @ w_h).  bv/bfc become an exact host-side
  constant row; nonzero bq reduces to a per-key bias on the exp.

Dtype strategy (empirically validated, rel_l2 ~6.5e-3 vs 2e-2 budget):
  * scores path in float8e4 (e4m3) with DoubleRow perf mode: each matmul
    contracts 2x128 rows at 0.5 PE-cycles per output column — 4x fewer PE
    cycles than fp32r.  wa is pre-scaled by 2^11 so fp8 sees well-scaled
    values; the exp activation applies scale=2^-11 to undo it.
  * v path (w = x@wc, ctx = probs@w) and probs in bfloat16: fp8 here fails
    the error budget (quantization error of probs/w lands directly on the
    output without averaging down).

Causal masking costs no vector work: each masked 128x128 block gets one extra
fp8 matmul accumulated into the scores PSUM (lhsT = -240*maskpattern,
rhs = 240*I adds -57600 -> exp gives ~6e-13, effectively 0).  A ones column
appended to w (width 257) makes the ctx matmul emit softmax denominators in
PSUM column 256 for free.  Normalize+accumulate across heads is a single
fused scalar_tensor_tensor (acc = ps*rec + acc) on the Pool engine.

The per-head program is software-pipelined: head h's ctx matmuls are
interleaved into head h+1's uT/w/scores phases so the PE never waits on the
exp (Activation) engine, and PSUM tile-pool rotation stalls are covered.
"""

import numpy as np
import ml_dtypes

import concourse.mybir as mybir
import concourse.tile as tile
from concourse import bacc
from concourse.bass_utils import run_bass_kernel_spmd

# Problem constants
B, S, E, H = 4, 1024, 256, 12
P = 128
NCORES = 8
HPC = H // 2            # heads per core
EH = E * HPC            # 1536 = per-core head width
KS_E = E // P           # 2 contraction subtiles over E
ST = S // P             # 8 row-blocks of S
CTXW = E + 1            # ctx matmul width incl. ones column
EWW = E + 2             # ww storage width (even, for alignment)

KA = 11                 # wa pre-scale exponent (fp8 dynamic range centering)
F8MAX = 240.0           # float8e4 (IEEE e4m3) max finite
MASKV = -F8MAX * F8MAX  # per-element additive mask in the scaled-score domain

F8 = mybir.dt.float8e4
BF = mybir.dt.bfloat16
F32 = mybir.dt.float32
NF8 = ml_dtypes.float8_e4m3
NBF = ml_dtypes.bfloat16
DR = mybir.MatmulPerfMode.DoubleRow

LAST_RESULTS = None     # BassKernelResults of the most recent run (for harness)


def _mask_structure(attention_mask):
    """Analyze the additive mask into per-key-block spans and block ops.

    Returns (struct, patterns, emuls):
      struct = (spans, covers, offs, tot, groups, blockops) hashable
      patterns: [nadd, P, P] float32 fp8-add patterns (-F8MAX where masked)
      emuls:    [nmul, P, P] float32 multiplicative exp(mask) blocks (rare path)
    """
    m = np.asarray(attention_mask, dtype=np.float64).reshape(S, S)   # [q, k]
    em = np.exp(m).astype(np.float32)
    emT = np.ascontiguousarray(em.T)                                 # [k, q]

    add_uniq: dict[bytes, int] = {}
    mul_uniq: dict[bytes, int] = {}
    blocks = {}
    for ki in range(ST):
        for qj in range(ST):
            blk = np.ascontiguousarray(emT[ki * P:(ki + 1) * P, qj * P:(qj + 1) * P])
            if not blk.any():
                blocks[(ki, qj)] = ("skip", 0)
            elif (blk == 1.0).all():
                blocks[(ki, qj)] = ("one", 0)
            elif ((blk == 0.0) | (blk == 1.0)).all():
                pat = (-F8MAX * (1.0 - blk.T)).astype(np.float32)    # [q, k]
                blocks[(ki, qj)] = ("add", add_uniq.setdefault(pat.tobytes(), len(add_uniq)))
            else:
                blocks[(ki, qj)] = ("mul", mul_uniq.setdefault(blk.tobytes(), len(mul_uniq)))

    zpat = (-F8MAX * np.ones((P, P), np.float32)).tobytes()
    spans = []
    blockops = []
    for ki in range(ST):
        non = [qj for qj in range(ST) if blocks[(ki, qj)][0] != "skip"]
        if not non:
            spans.append(None)
            blockops.append(())
            continue
        qa, qb = non[0] * P, (non[-1] + 1) * P
        spans.append((qa, qb))
        ops = []
        for qj in range(qa // P, qb // P):
            kind, uid = blocks[(ki, qj)]
            if kind == "one":
                continue
            if kind == "skip":  # interior hole: mask it with the all-masked pattern
                kind, uid = "add", add_uniq.setdefault(zpat, len(add_uniq))
            ops.append((qj, kind, uid))
        blockops.append(tuple(ops))

    offs, tot = [], 0
    for ki in range(ST):
        offs.append(tot)
        if spans[ki] is not None:
            tot += spans[ki][1] - spans[ki][0]

    covers = []
    for m_ in range(ST):
        ks = tuple(ki for ki in range(ST)
                   if spans[ki] is not None
                   and spans[ki][0] <= m_ * P and spans[ki][1] >= (m_ + 1) * P)
        assert ks, (
            "attention row-block with no unmasked keys is not supported "
            "(reference softmax of an all-masked row is uniform)")
        covers.append(ks)

    # exp groups: greedy consecutive-ki packing, total width <= 1024 (2 banks)
    groups = []
    curg, curw = [], 0
    for ki in range(ST):
        if spans[ki] is None:
            continue
        w = spans[ki][1] - spans[ki][0]
        if curg and curw + w > 1024:
            groups.append((tuple(curg), curw))
            curg, curw = [], 0
        curg.append(ki)
        curw += w
    if curg:
        groups.append((tuple(curg), curw))

    nadd = max(len(add_uniq), 1)
    patterns = np.zeros((nadd, P, P), np.float32)
    for key, uid in add_uniq.items():
        patterns[uid] = np.frombuffer(key, np.float32).reshape(P, P)
    nmul = max(len(mul_uniq), 1)
    emuls = np.ones((nmul, P, P), np.float32)
    for key, uid in mul_uniq.items():
        emuls[uid] = np.frombuffer(key, np.float32).reshape(P, P)

    struct = (tuple(spans), tuple(covers), tuple(offs), tot, tuple(groups),
              tuple(blockops), len(add_uniq) > 0, len(mul_uniq) > 0)
    return struct, patterns, emuls


def _build(struct, has_qk_bias):
    spans, covers, offs, tot, groups, blockops, has_add, has_mul = struct
    Exp = mybir.ActivationFunctionType.Exp
    MULT = mybir.AluOpType.mult
    ADD = mybir.AluOpType.add

    nc = bacc.Bacc("TRN2")
    xT8_d = nc.dram_tensor("xT8", (E, S), F8, kind="ExternalInput")
    xTbf_d = nc.dram_tensor("xTbf", (E, S), BF, kind="ExternalInput")
    wa8_d = nc.dram_tensor("wa8", (E, EH), F8, kind="ExternalInput")
    wcbf_d = nc.dram_tensor("wcbf", (E, EH), BF, kind="ExternalInput")
    nadd_shape = max([uid + 1 for ops in blockops for (qj, k, uid) in ops if k == "add"] + [1])
    nmul_shape = max([uid + 1 for ops in blockops for (qj, k, uid) in ops if k == "mul"] + [1])
    mpat_d = nc.dram_tensor("mpat", (nadd_shape, P, P), F8, kind="ExternalInput")
    cdiag_d = nc.dram_tensor("cdiag", (P, P), F8, kind="ExternalInput")
    emul_d = nc.dram_tensor("emul", (nmul_shape, P, P), BF, kind="ExternalInput")
    wm2_d = nc.dram_tensor("wm2", (E, 2 * HPC), BF, kind="ExternalInput")
    y_d = nc.dram_tensor("y", (S, E), F32, kind="ExternalOutput")

    # Engine roles (GPSIMD/Pool cannot touch PSUM on TRN2):
    #   PE   — all matmuls (incl. fp8 mask-add blocks)
    #   Act  — exp over merged score spans; uT PSUM->SBUF fp8 copies
    #   DVE  — ww PSUM->SBUF bf16 copies; reciprocal; fused normalize+acc
    #   Pool — SBUF-only memsets (ones columns)
    with tile.TileContext(nc) as tc, \
            tc.tile_pool(name="singles", bufs=1) as singles, \
            tc.tile_pool(name="heads", bufs=2) as heads, \
            tc.tile_pool(name="small", bufs=4) as small, \
            tc.tile_pool(name="psS", bufs=2, space="PSUM") as psS, \
            tc.tile_pool(name="psA", bufs=2, space="PSUM") as psA, \
            tc.tile_pool(name="psC", bufs=2, space="PSUM") as psC:

        # ---- resident tensors, DMA'd in first-use order ----
        wa8_sb = singles.tile([P, KS_E, EH], F8)
        xT8_sb = singles.tile([P, KS_E, S], F8)
        xTbf_sb = singles.tile([P, KS_E, S], BF)
        wcbf_sb = singles.tile([P, KS_E, EH], BF)
        cdiag_sb = singles.tile([P, P], F8)
        mpat_sb = singles.tile([P, nadd_shape, P], F8)
        h0 = slice(0, E)
        nc.sync.dma_start(wa8_sb[:, :, h0],
                          wa8_d[:, h0].rearrange("(ko p) n -> p ko n", p=P))
        nc.sync.dma_start(xT8_sb, xT8_d[:, :].rearrange("(ko p) n -> p ko n", p=P))
        nc.sync.dma_start(cdiag_sb, cdiag_d[:, :])
        nc.sync.dma_start(mpat_sb, mpat_d[:, :, :].rearrange("u p q -> p u q"))
        nc.sync.dma_start(xTbf_sb, xTbf_d[:, :].rearrange("(ko p) n -> p ko n", p=P))
        nc.sync.dma_start(wcbf_sb[:, :, h0],
                          wcbf_d[:, h0].rearrange("(ko p) n -> p ko n", p=P))
        hrest = slice(E, EH)
        nc.sync.dma_start(wa8_sb[:, :, hrest],
                          wa8_d[:, hrest].rearrange("(ko p) n -> p ko n", p=P))
        nc.sync.dma_start(wcbf_sb[:, :, hrest],
                          wcbf_d[:, hrest].rearrange("(ko p) n -> p ko n", p=P))
        emul_sb = None
        if has_mul:
            emul_sb = singles.tile([P, nmul_shape, P], BF)
            nc.sync.dma_start(emul_sb, emul_d[:, :, :].rearrange("u p q -> p u q"))
        wm2_sb = None
        if has_qk_bias:
            wm2_sb = singles.tile([P, KS_E, 2 * HPC], BF)
            nc.sync.dma_start(
                wm2_sb, wm2_d[:, :].rearrange("(ko p) n -> p ko n", p=P))
        acc_sb = singles.tile([P, ST, E], F32)

        prev = None   # (h, uT8, ww, probs) of the previous head

        def emit_ctx_unit(ph, m_, probs_t, ww_t):
            """ctx for head ph, q-block m_: matmuls + reciprocal + fused
            normalize-accumulate; final head DMAs the output row-block."""
            psc = psC.tile([P, CTXW], F32, tag="ctx", name="ps_c")
            ks_list = covers[m_]
            last = len(ks_list) - 1
            for idx, ki in enumerate(ks_list):
                qa, _ = spans[ki]
                c0 = offs[ki] + m_ * P - qa
                nc.tensor.matmul(
                    psc,
                    probs_t[:, c0:c0 + P],
                    ww_t[:, ki, 0:CTXW],
                    start=(idx == 0), stop=(idx == last),
                )
            rec = small.tile([P, 1], F32, tag="rec")
            nc.vector.reciprocal(rec, psc[:, E:E + 1])
            if ph == 0:
                nc.vector.tensor_scalar_mul(acc_sb[:, m_, :], psc[:, 0:E], rec)
            else:
                nc.vector.scalar_tensor_tensor(
                    acc_sb[:, m_, :], psc[:, 0:E], rec, acc_sb[:, m_, :],
                    MULT, ADD)
            if ph == HPC - 1:
                nc.sync.dma_start(y_d[m_ * P:(m_ + 1) * P, :], acc_sb[:, m_, :])

        ww_pair = None
        for h in range(HPC):
            uT8 = heads.tile([P, KS_E, S], F8, tag="uT")
            probs = heads.tile([P, tot], BF, tag="probs")
            ctx_queue = list(range(ST)) if prev is not None else []

            def pull(n):
                for _ in range(n):
                    if not ctx_queue:
                        return
                    emit_ctx_unit(prev[0], ctx_queue.pop(0), prev[1], prev[2])

            # ---- uT = wa^T @ xT, fp8 DoubleRow (contraction 256 per mm);
            #      both e-tiles share one 2-bank PSUM -> one wide Act copy ----
            for jn in range(2):
                ps = psS.tile([P, 1024], F32, tag="scores", name="ps_u")
                for t in range(KS_E):
                    nc.tensor.matmul(
                        ps[:, t * 512:(t + 1) * 512],
                        wa8_sb[:, :, h * E + t * P: h * E + (t + 1) * P],
                        xT8_sb[:, :, jn * 512:(jn + 1) * 512],
                        start=True, stop=True, perf_mode=DR,
                    )
                    pull(1)
                nc.scalar.copy(
                    uT8[:, 0:KS_E, jn * 512:(jn + 1) * 512],
                    ps.rearrange("p (t n) -> p t n", t=KS_E))

            # ---- s2_h = x @ wm2_h: per-key exp bias (only if bq != 0) ----
            s2 = None
            if has_qk_bias:
                s2 = heads.tile([P, ST, 2], F32, tag="s2")
                for st in range(ST):
                    ps = psA.tile([P, 512], F32, tag="mm512", name="ps_s2")[:, :2]
                    for ks in range(KS_E):
                        nc.tensor.matmul(
                            ps,
                            xTbf_sb[:, ks, st * P:(st + 1) * P],
                            wm2_sb[:, ks, 2 * h:2 * h + 2],
                            start=(ks == 0), stop=(ks == KS_E - 1),
                        )
                    nc.vector.tensor_copy(s2[:, st, :], ps)

            # ---- w = x @ wc for the head PAIR (h, h+1), bf16 N=512 ----
            if h % 2 == 0:
                ww_pair = heads.tile([P, ST, 2, EWW], BF, tag="wwp")
                nc.gpsimd.memset(ww_pair[:, :, :, E:EWW], 1.0)
                for st in range(ST):
                    ps = psA.tile([P, 512], F32, tag="mm512", name="ps_w")
                    for ks in range(KS_E):
                        nc.tensor.matmul(
                            ps,
                            xTbf_sb[:, ks, st * P:(st + 1) * P],
                            wcbf_sb[:, ks, h * E:(h + 2) * E],
                            start=(ks == 0), stop=(ks == KS_E - 1),
                        )
                    nc.vector.tensor_copy(
                        ww_pair[:, st, :, 0:E],
                        ps.rearrange("p (a n) -> p a n", a=2))
                    if st % 2 == 1:
                        pull(1)
            ww = ww_pair[:, :, h % 2, :]

            # ---- scoresT -> (+mask adds) -> exp => probs (packed bf16) ----
            for gi, (kis, gw) in enumerate(groups):
                pss = psS.tile([P, 1024], F32, tag="scores", name="ps_s")
                goff = offs[kis[0]]
                # per-2KB-bank accumulation chains: score chunks then mask adds
                items = {0: [], 1: []}
                for ki in kis:
                    qa, qb = spans[ki]
                    o = offs[ki] - goff
                    pos = o
                    while pos < o + (qb - qa):
                        end = min(o + (qb - qa), (pos // 512 + 1) * 512)
                        items[pos // 512].append(("chunk", ki, pos, end, 0))
                        pos = end
                for ki in kis:
                    qa, _ = spans[ki]
                    o = offs[ki] - goff
                    for (qj, kind, uid) in blockops[ki]:
                        if kind != "add":
                            continue
                        c = o + qj * P - qa
                        items[c // 512].append(("mask", ki, c, c + P, uid))
                for bank in (0, 1):
                    blist = items[bank]
                    for idx, (kind, ki, c0, c1, uid) in enumerate(blist):
                        first, lastb = idx == 0, idx == len(blist) - 1
                        if kind == "chunk":
                            qa, _ = spans[ki]
                            o = offs[ki] - goff
                            pos0 = qa + (c0 - o)
                            nc.tensor.matmul(
                                pss[:, c0:c1],
                                uT8[:, :, ki * P:(ki + 1) * P],
                                xT8_sb[:, :, pos0:pos0 + (c1 - c0)],
                                start=first, stop=lastb, perf_mode=DR,
                            )
                        else:
                            nc.tensor.matmul(
                                pss[:, c0:c1],
                                mpat_sb[:, uid, :],
                                cdiag_sb,
                                start=first, stop=lastb,
                            )
                if not has_qk_bias:
                    nc.scalar.activation(
                        probs[:, goff:goff + gw], pss[:, 0:gw], Exp,
                        scale=2.0 ** -KA)
                else:
                    for ki in kis:
                        qa, qb = spans[ki]
                        o = offs[ki] - goff
                        nc.scalar.activation(
                            probs[:, offs[ki]:offs[ki] + qb - qa],
                            pss[:, o:o + qb - qa], Exp,
                            scale=2.0 ** -KA, bias=s2[:, ki, 0:1])
                # rare general-mask path: multiplicative blocks
                for ki in kis:
                    qa, _ = spans[ki]
                    for (qj, kind, uid) in blockops[ki]:
                        if kind != "mul":
                            continue
                        sl = slice(offs[ki] + qj * P - qa,
                                   offs[ki] + (qj + 1) * P - qa)
                        nc.vector.tensor_mul(probs[:, sl], probs[:, sl],
                                             emul_sb[:, uid, :])
                if gi % 2 == 1:
                    pull(1)
            pull(ST)   # drain any ctx units not yet emitted

            prev = (h, probs, ww)

        # tail: ctx of the final head
        for m_ in range(ST):
            emit_ctx_unit(prev[0], m_, prev[1], prev[2])

    nc.compile()
    return nc


_nc_cache = {}


def make_core_inputs(x, attention_mask, Wq, bq, Wk, bk, Wv, bv, Wfc, bfc):
    """Host-side prep shared by kernel() and test harnesses.

    Returns (struct, has_qk_bias, in_maps, ybias).
    """
    x = np.asarray(x, np.float32)
    Wq64 = np.asarray(Wq, np.float64)
    Wk64 = np.asarray(Wk, np.float64)
    Wv64 = np.asarray(Wv, np.float64)
    Wfc64 = np.asarray(Wfc, np.float64)
    bq64 = np.asarray(bq, np.float64)
    bv64 = np.asarray(bv, np.float64)
    bfc64 = np.asarray(bfc, np.float64)

    has_qk_bias = bool(bq64.any())
    struct, patterns, emuls = _mask_structure(attention_mask)

    scale = 1.0 / np.sqrt(np.float64(E))
    wa = np.empty((E, E * H), np.float32)
    wc = np.empty((E, E * H), np.float32)
    wm2 = np.zeros((E, 2 * H), np.float32)
    for g in range(H):
        gs = slice(g * E, (g + 1) * E)
        wa[:, gs] = np.clip(scale * (Wk64[:, gs] @ Wq64[:, gs].T) * 2.0 ** KA,
                            -F8MAX, F8MAX).astype(np.float32)
        wc[:, gs] = (Wv64[:, gs] @ Wfc64[gs, :]).astype(np.float32)
        wm2[:, 2 * g] = (scale * (Wk64[:, gs] @ bq64[gs])).astype(np.float32)
    ybias = (bv64 @ Wfc64 + bfc64).astype(np.float32)

    cdiag = (F8MAX * np.eye(P, dtype=np.float32)).astype(NF8)
    mpat8 = patterns.astype(NF8)
    emulbf = emuls.astype(NBF)

    in_maps = []
    for c in range(NCORES):
        b, hg = divmod(c, 2)
        cs = slice(hg * EH, (hg + 1) * EH)
        xT = np.ascontiguousarray(x[b].T)
        in_maps.append({
            "xT8": np.clip(xT, -F8MAX, F8MAX).astype(NF8),
            "xTbf": xT.astype(NBF),
            "wa8": np.ascontiguousarray(wa[:, cs]).astype(NF8),
            "wcbf": np.ascontiguousarray(wc[:, cs]).astype(NBF),
            "mpat": mpat8,
            "cdiag": cdiag,
            "emul": emulbf,
            "wm2": np.ascontiguousarray(
                wm2[:, hg * 2 * HPC:(hg + 1) * 2 * HPC]).astype(NBF),
        })
    return struct, has_qk_bias, in_maps, ybias


def kernel(x, attention_mask, Wq, bq, Wk, bk, Wv, bv, Wfc, bfc, _trace=False):
    global LAST_RESULTS
    struct, has_qk_bias, in_maps, ybias = make_core_inputs(
        x, attention_mask, Wq, bq, Wk, bk, Wv, bv, Wfc, bfc)
    key = (struct, has_qk_bias, KA)
    if key not in _nc_cache:
        _nc_cache[key] = _build(struct, has_qk_bias)
    nc = _nc_cache[key]

    from concourse._compat import axon_active
    if axon_active() and not _trace:
        results = _run_pjrt_cached(key, nc, in_maps)
        LAST_RESULTS = None
    else:
        try:
            res = run_bass_kernel_spmd(nc, in_maps, core_ids=list(range(NCORES)),
                                       trace=_trace)
        except ModuleNotFoundError:
            # axon client without NTFF-profiling support: tracing disabled
            import os
            os.environ["BASS_NEVER_TRACE"] = "1"
            res = run_bass_kernel_spmd(nc, in_maps, core_ids=list(range(NCORES)),
                                       trace=False)
        LAST_RESULTS = res
        results = res.results
    out = np.empty((B, S, E), np.float32)
    for b in range(B):
        out[b] = results[2 * b]["y"] + results[2 * b + 1]["y"] + ybias
    return out


_jit_cache = {}


def _run_pjrt_cached(key, nc, in_maps):
    """bass2jax.run_bass_via_pjrt with the sharded jit cached per kernel
    structure, so repeated kernel() calls skip re-tracing (and with it the
    expensive NEFF recompile inside the neuronx_cc hook)."""
    import jax
    from jax.sharding import Mesh, PartitionSpec
    from jax.experimental.shard_map import shard_map
    from concourse import bass2jax
    import concourse.mybir as _mybir

    if key not in _jit_cache:
        bass2jax.install_neuronx_cc_hook()
        in_names, out_names, out_avals, zero_shapes = [], [], [], []
        for alloc in nc.m.functions[0].allocations:
            if not isinstance(alloc, _mybir.MemoryLocationSet):
                continue
            name = alloc.memorylocations[0].name
            if alloc.kind == "ExternalInput":
                if name != "partition_id":
                    in_names.append(name)
            elif alloc.kind == "ExternalOutput":
                shape = tuple(alloc.tensor_shape)
                dtype = _mybir.dt.np(alloc.dtype)
                out_names.append(name)
                out_avals.append(jax.core.ShapedArray(shape, dtype))
                zero_shapes.append((shape, dtype))
        n_params = len(in_names)
        n_outs = len(out_names)
        all_names = in_names + out_names + ["partition_id"]

        def _body(*args):
            operands = list(args)
            operands.append(bass2jax.partition_id_tensor())
            return tuple(bass2jax._bass_exec_p.bind(
                *operands,
                out_avals=tuple(out_avals),
                in_names=tuple(all_names),
                out_names=tuple(out_names),
                lowering_input_output_aliases=(),
                sim_require_finite=True,
                sim_require_nnan=True,
                nc=nc,
            ))

        devices = jax.devices()[:NCORES]
        mesh = Mesh(np.asarray(devices), ("core",))
        sharded = jax.jit(
            shard_map(_body, mesh=mesh,
                      in_specs=(PartitionSpec("core"),) * (n_params + n_outs),
                      out_specs=(PartitionSpec("core"),) * n_outs,
                      check_rep=False),
            donate_argnums=tuple(range(n_params, n_params + n_outs)),
            keep_unused=True,
        )
        _jit_cache[key] = (sharded, in_names, out_names, out_avals, zero_shapes)

    sharded, in_names, out_names, out_avals, zero_shapes = _jit_cache[key]
    concat_in = [
        np.concatenate([np.asarray(m[name]) for m in in_maps], axis=0)
        for name in in_names
    ]

    def _exec():
        concat_zeros = [np.zeros((NCORES * s[0], *s[1:]), d)
                        for s, d in zero_shapes]
        out_arrs = sharded(*concat_in, *concat_zeros)
        return [np.asarray(a) for a in out_arrs]

    try:
        out_arrs = _exec()
    except Exception:
        # transient device/transport flake: drop the failed call's effect
        # tokens (else jax's atexit block_until_ready re-raises even after a
        # successful retry) and retry once with fresh buffers
        try:
            from jax._src import dispatch as _jd
            _jd.runtime_tokens.clear()
        except Exception:
            pass
        out_arrs = _exec()
    return [
        {name: out_arrs[i].reshape(NCORES, *out_avals[i].shape)[c]
         for i, name in enumerate(out_names)}
        for c in range(NCORES)
    ]
